# revision 1
# baseline (speedup 1.0000x reference)
"""Trainium2 Bass kernel for DeepSet MLP (embedding-lookup-sum + 3-layer MLP).

Math: u[b] = sum_j W_phi[x[b,j]] + N*b_phi
      y[b] = relu(relu(u@W1+b1)@W2+b2)@W3 + b3

Instead of gathering B*N embedding rows (1 GiB of traffic), each core
computes per-row class histograms and contracts them with the table:
    u = counts @ W_phi,  counts[b,c] = #{j : x[b,j]=c}
The histogram is built on the PE via a class split c = 32*hi + lo:
one-hot H (32 lo-classes) and G (16 hi-classes) per token, then per row
cnt2[b] = H_b^T @ G_b (one matmul per row, j contracted on partitions;
the 4 j-chunk partials land in the block-diagonal of the output and are
summed inside the projection matmul via 4x-replicated weights).
The projection u = cnt2 @ W_phi (bf16x2 for fp32-level accuracy) and the
MLP run on the PE as well.

Data-parallel: batch 4096 sharded 512 rows per core across 8 cores.
"""

import os
import numpy as np
from contextlib import ExitStack

import concourse.bass as bass
import concourse.bacc as bacc
import concourse.tile as tile
import concourse.mybir as mybir
from concourse import masks
from concourse.bass_utils import run_bass_kernel_spmd

B, N, C, PHI = 4096, 512, 512, 128
H1, H2 = 512, 256
NCORES = 8
BS = B // NCORES          # 512 batch rows per core
NB = BS // 128            # 4 batch blocks of 128 rows
NJ = N // 128             # 4 j-chunks
LO, HI = 32, 16           # class split: c = 32*hi + lo

F32 = mybir.dt.float32
BF16 = mybir.dt.bfloat16
I16 = mybir.dt.int16
I32 = mybir.dt.int32
AF = mybir.ActivationFunctionType
ALU = mybir.AluOpType

STAGE = int(os.environ.get("K_STAGE", "99"))  # debug: stop after stage N


def build_program():
    nc = bacc.Bacc("TRN2", target_bir_lowering=False, debug=False,
                   num_devices=NCORES)

    x32 = nc.dram_tensor("x", [BS, N], I32, kind="ExternalInput")
    wphi = nc.dram_tensor("wphi", [C, PHI], F32, kind="ExternalInput")
    bphi = nc.dram_tensor("bphi", [PHI, 1], F32, kind="ExternalInput")
    w1 = nc.dram_tensor("w1", [PHI, H1], F32, kind="ExternalInput")
    b1 = nc.dram_tensor("b1", [PHI, H1 // PHI], F32, kind="ExternalInput")
    w2 = nc.dram_tensor("w2", [H1, H2], F32, kind="ExternalInput")
    b2 = nc.dram_tensor("b2", [PHI, H2 // PHI], F32, kind="ExternalInput")
    w3 = nc.dram_tensor("w3", [PHI, H2 // PHI], F32, kind="ExternalInput")
    b3 = nc.dram_tensor("b3", [1, 1], F32, kind="ExternalInput")
    out = nc.dram_tensor("out", [1, BS], F32, kind="ExternalOutput")

    with tile.TileContext(nc) as tc:
        with ExitStack() as ctx:
            _emit(ctx, tc, nc, x32, wphi, bphi, w1, b1, w2, b2, w3, b3, out)
    nc.compile()
    return nc


def _emit(ctx, tc, nc, x32, wphi, bphi, w1, b1, w2, b2, w3, b3, out):
    consts = ctx.enter_context(tc.tile_pool(name="consts", bufs=1))
    xin = ctx.enter_context(tc.tile_pool(name="xin", bufs=2))
    xtp = ctx.enter_context(tc.tile_pool(name="xtp", bufs=1))
    eqp = ctx.enter_context(tc.tile_pool(name="eqp", bufs=2))
    fp = ctx.enter_context(tc.tile_pool(name="fp", bufs=1))
    mlp = ctx.enter_context(tc.tile_pool(name="mlp", bufs=1))
    ps_cnt = ctx.enter_context(tc.tile_pool(name="ps_cnt", bufs=2, space="PSUM"))
    ps_u = ctx.enter_context(tc.tile_pool(name="ps_u", bufs=1, space="PSUM"))
    ps_mlp = ctx.enter_context(tc.tile_pool(name="ps_mlp", bufs=2, space="PSUM"))
    ps_y = ctx.enter_context(tc.tile_pool(name="ps_y", bufs=1, space="PSUM"))

    ident = consts.tile([128, 128], F32)
    masks.make_identity(nc, ident[:])

    # ---- weights / biases to SBUF ----
    # wphiP: W_phi replicated 4x along partitions: partition (i*32+r) holds
    # row W_phi[h*32+r] at free slot h (h=hi class). The projection matmul
    # contracts all 128 partitions at once, summing the 4 j-chunk partials.
    wphiP = consts.tile([128, HI * PHI], F32)
    wsrc = wphi.ap().rearrange("(h r) d -> r h d", r=32)
    for i in range(4):
        nc.sync.dma_start(wphiP[32 * i:32 * (i + 1), :], wsrc)
    # bf16x2 decomposition of the table for exact-ish bf16 matmuls
    wphiH = consts.tile([128, HI * PHI], BF16)
    wphiL = consts.tile([128, HI * PHI], BF16)
    wres = consts.tile([128, HI * PHI], F32)
    nc.vector.tensor_copy(wphiH[:], wphiP[:])
    nc.vector.tensor_tensor(out=wres[:], in0=wphiP[:], in1=wphiH[:],
                            op=ALU.subtract)
    nc.vector.tensor_copy(wphiL[:], wres[:])

    bphi_sb = consts.tile([128, 1], F32)
    nc.sync.dma_start(bphi_sb[:], bphi.ap())
    bphiN = consts.tile([128, 1], F32)
    nc.vector.tensor_scalar(out=bphiN[:], in0=bphi_sb[:], scalar1=float(N),
                            scalar2=None, op0=ALU.mult)

    w1sb = consts.tile([128, H1], F32)
    nc.sync.dma_start(w1sb[:], w1.ap())
    b1sb = consts.tile([128, 4], F32)
    nc.sync.dma_start(b1sb[:], b1.ap())
    w2sb = consts.tile([128, 4 * H2], F32)
    nc.sync.dma_start(w2sb[:], w2.ap().rearrange("(c p) h -> p c h", p=128))
    b2sb = consts.tile([128, 2], F32)
    nc.sync.dma_start(b2sb[:], b2.ap())
    w3sb = consts.tile([128, 2], F32)
    nc.sync.dma_start(w3sb[:], w3.ap())
    b3sb = consts.tile([1, 1], F32)
    nc.sync.dma_start(b3sb[:], b3.ap())

    # ---- index staging ----
    # xiT/xhiT/xloT: [j, (bb, jc, b)] so each block's slice is contiguous
    xiT = xtp.tile([128, NB * NJ * 128], I16)
    xhiT = xtp.tile([128, NB * NJ * 128], I16)
    xloT = xtp.tile([128, NB * NJ * 128], I16)
    # F: per-row joint counts, partition (i*32+lo), free (hi, b)
    fcnt = fp.tile([128, HI * BS], BF16)

    usb = mlp.tile([128, BS], F32)
    h1sb = [mlp.tile([128, BS], F32, tag=f"h1_{k}", name=f"h1sb{k}")
            for k in range(4)]
    h2sb = [mlp.tile([128, BS], F32, tag=f"h2_{k}", name=f"h2sb{k}")
            for k in range(2)]
    ysb = mlp.tile([1, BS], F32)

    def dbg_out(src_f32_row):
        nc.vector.tensor_copy(ysb[:], src_f32_row)
        nc.sync.dma_start(out.ap(), ysb[:])

    if STAGE == 0:
        t0 = mlp.tile([1, BS], F32, name="dbg0")
        nc.vector.tensor_copy(t0[:], wphiP[0:1, 0:BS])
        dbg_out(t0[:])
        return

    for bb in range(NB):
        # --- stage A: load 128 rows, cast, transpose, split hi/lo ---
        xrows = xin.tile([128, N], I32, tag="xrows")
        nc.sync.dma_start(xrows[:], x32.ap()[bb * 128:(bb + 1) * 128, :])
        xf = xin.tile([128, N], F32, tag="xf")
        nc.vector.tensor_copy(xf[:], xrows[:])
        for jc in range(NJ):
            pst = ps_mlp.tile([128, BS], F32, tag="ph", name="pst")
            nc.tensor.transpose(pst[:, 0:128], xf[:, jc * 128:(jc + 1) * 128],
                                ident[:])
            col = (bb * NJ + jc) * 128
            nc.vector.tensor_copy(xiT[:, col:col + 128], pst[:, 0:128])
        blk = slice(bb * 512, (bb + 1) * 512)
        nc.vector.tensor_scalar(out=xhiT[:, blk], in0=xiT[:, blk], scalar1=5,
                                scalar2=None, op0=ALU.logical_shift_right)
        nc.vector.tensor_scalar(out=xloT[:, blk], in0=xiT[:, blk], scalar1=31,
                                scalar2=None, op0=ALU.bitwise_and)
        if STAGE == 1:
            t1 = mlp.tile([1, BS], F32, name="dbg1")
            nc.vector.tensor_copy(t1[:], xloT[0:1, :BS])
            dbg_out(t1[:])
            return

        # --- stage B: one-hots via is_equal ---
        # H2 [j, (jc, lo, b)]  G2 [j, (jc, hi, b)] — per-row matmul operand
        # slices are then single stride-128 runs (walrus requires matmul
        # APs with one free dim), eq writes keep a packed innermost dim.
        h2t = eqp.tile([128, LO * NJ * 128], BF16, tag="h2t")
        g2t = eqp.tile([128, HI * NJ * 128], BF16, tag="g2t")
        h2v = h2t[:].rearrange("p (jc l b) -> p jc l b", jc=NJ, l=LO)
        g2v = g2t[:].rearrange("p (jc h b) -> p jc h b", jc=NJ, h=HI)
        for lo in range(LO):
            nc.vector.tensor_scalar(out=h2v[:, :, lo:lo + 1, :],
                                    in0=xloT[:, blk], scalar1=lo,
                                    scalar2=None, op0=ALU.is_equal)
        for hi in range(HI):
            nc.vector.tensor_scalar(out=g2v[:, :, hi:hi + 1, :],
                                    in0=xhiT[:, blk], scalar1=hi,
                                    scalar2=None, op0=ALU.is_equal)
        if STAGE == 2:
            t2 = mlp.tile([1, BS], F32, name="dbg2")
            nc.vector.tensor_copy(t2[:], h2t[0:1, :BS])
            dbg_out(t2[:])
            return

        # --- stage C: per-row count matmuls ---
        # one matmul per row: lhsT [j, (jc, lo)] (m=128), rhs [j, (jc', hi)]
        # (n=64); diagonal jc==jc' blocks of out hold the counts.
        h2m = h2t[:].rearrange("p (m b) -> p m b", b=128)
        g2m = g2t[:].rearrange("p (m b) -> p m b", b=128)
        fv = fcnt[:].rearrange("p (h b) -> p h b", h=HI)
        for k16 in range(8):            # 8 psum tiles of 16 rows each
            pc = ps_cnt.tile([128, 1024], F32)
            for s in range(16):
                b_l = k16 * 16 + s
                nc.tensor.matmul(
                    pc[:, s * 64:(s + 1) * 64],
                    h2m[:, :, b_l:b_l + 1],
                    g2m[:, :, b_l:b_l + 1],
                    start=True, stop=True)
            # evacuate diagonal blocks to F (ACT, Copy only -> no table swaps)
            pcv = pc[:].rearrange("p (s i h) -> p s i h", s=16, i=NJ)
            b0 = bb * 128 + k16 * 16
            for i in range(NJ):
                src = pcv[32 * i:32 * (i + 1), :, i:i + 1, :]
                dst = fv[32 * i:32 * (i + 1), :, b0:b0 + 16]
                nc.scalar.copy(dst.transpose([0, 2, 1]), src)

    if STAGE == 3:
        t3 = mlp.tile([1, BS], F32, name="dbg3")
        nc.vector.tensor_copy(t3[:], fcnt[0:1, :BS])
        dbg_out(t3[:])
        return

    # ---- projection u_T[d, b] = sum_c counts_T[c, b] * W_phi[c, d] ----
    # Weights replicated across the 4 partition blocks: one k=128 matmul per
    # hi-class sums over lo-classes and the 4 j-chunk partials.
    pu = ps_u.tile([128, BS], F32)
    k = 0
    for h in range(HI):
        for w in (wphiH, wphiL):
            nc.tensor.matmul(
                pu[:], w[:, PHI * h:PHI * (h + 1)], fv[:, h, :],
                start=(k == 0), stop=(k == 2 * HI - 1))
            k += 1
    nc.vector.tensor_scalar(out=usb[:], in0=pu[:], scalar1=bphiN[:, 0:1],
                            scalar2=None, op0=ALU.add)
    if STAGE == 4:
        dbg_out(usb[0:1, :])
        return

    # ---- MLP ----
    for hc in range(4):
        ph = ps_mlp.tile([128, BS], F32, tag="ph", name="ph_a")
        nc.tensor.matmul(ph[:], w1sb[:, hc * 128:(hc + 1) * 128], usb[:],
                         start=True, stop=True)
        nc.scalar.activation(h1sb[hc][:], ph[:], AF.Relu,
                             bias=b1sb[:, hc:hc + 1], scale=1.0)
    w2v = w2sb[:].rearrange("p (c h) -> p c h", c=4)
    for mc in range(2):
        ph = ps_mlp.tile([128, BS], F32, tag="ph", name="ph_b")
        for kc in range(4):
            nc.tensor.matmul(ph[:], w2v[:, kc, mc * 128:(mc + 1) * 128],
                             h1sb[kc][:], start=(kc == 0), stop=(kc == 3))
        nc.scalar.activation(h2sb[mc][:], ph[:], AF.Relu,
                             bias=b2sb[:, mc:mc + 1], scale=1.0)
    py = ps_y.tile([1, BS], F32)
    for kc in range(2):
        nc.tensor.matmul(py[:], w3sb[:, kc:kc + 1], h2sb[kc][:],
                         start=(kc == 0), stop=(kc == 1))
    nc.vector.tensor_scalar(out=ysb[:], in0=py[:], scalar1=b3sb[0:1, 0:1],
                            scalar2=None, op0=ALU.add)
    nc.sync.dma_start(out.ap(), ysb[:])


_CACHED_NC = None


def _get_nc():
    global _CACHED_NC
    if _CACHED_NC is None:
        _CACHED_NC = build_program()
    return _CACHED_NC


def _prep_in_maps(x, W_phi, b_phi, W1, b1, W2, b2, W3, b3):
    x = np.ascontiguousarray(np.asarray(x, dtype=np.int64).astype(np.int32))
    W_phi = np.asarray(W_phi, dtype=np.float32)
    W1 = np.asarray(W1, dtype=np.float32)
    W2 = np.asarray(W2, dtype=np.float32)
    shared = {
        "wphi": W_phi,
        "bphi": np.asarray(b_phi, dtype=np.float32).reshape(PHI, 1),
        "w1": W1,
        "b1": np.ascontiguousarray(
            np.asarray(b1, np.float32).reshape(4, 128).T),
        "w2": W2,
        "b2": np.ascontiguousarray(
            np.asarray(b2, np.float32).reshape(2, 128).T),
        "w3": np.ascontiguousarray(
            np.asarray(W3, np.float32).reshape(2, 128).T),
        "b3": np.asarray(b3, np.float32).reshape(1, 1),
    }
    return [dict(shared, x=np.ascontiguousarray(x[c * BS:(c + 1) * BS]))
            for c in range(NCORES)]


def run(trace=False, **inputs):
    nc = _get_nc()
    in_maps = _prep_in_maps(**inputs)
    res = run_bass_kernel_spmd(nc, in_maps, core_ids=list(range(NCORES)),
                               trace=trace)
    y = np.concatenate([np.asarray(res.results[c]["out"]).reshape(BS)
                        for c in range(NCORES)])
    return y.reshape(B, 1).astype(np.float32), res


def kernel(**inputs):
    y, _ = run(trace=False, **inputs)
    return y



# revision 2
# speedup vs baseline: 5.3023x; 5.3023x over previous
"""Trainium2 Bass kernel for DeepSet MLP (embedding-lookup-sum + 3-layer MLP).

Math: u[b] = sum_j W_phi[x[b,j]] + N*b_phi
      y[b] = relu(relu(u@W1+b1)@W2+b2)@W3 + b3

Instead of gathering B*N embedding rows (1 GiB of traffic), each core
computes per-row class histograms and contracts them with the table:
    u = counts @ W_phi,  counts[b,c] = #{j : x[b,j]=c}
The histogram is built on the PE via a class split c = 32*hi + lo:
one-hot H (32 lo-classes) and G (16 hi-classes) per token, then per row
cnt2[b] = H_b^T @ G_b (one matmul per row, j contracted on partitions;
the 4 j-chunk partials land in the block-diagonal of the output and are
summed inside the projection matmul via 4x-replicated weights).
The projection u = cnt2 @ W_phi (bf16x2 for fp32-level accuracy) and the
MLP run on the PE as well.

Data-parallel: batch 4096 sharded 512 rows per core across 8 cores.
"""

import os
import numpy as np
from contextlib import ExitStack

import concourse.bass as bass
import concourse.bacc as bacc
import concourse.tile as tile
import concourse.mybir as mybir
from concourse import masks
from concourse.bass_utils import run_bass_kernel_spmd

B, N, C, PHI = 4096, 512, 512, 128
H1, H2 = 512, 256
NCORES = 8
BS = B // NCORES          # 512 batch rows per core
NB = BS // 128            # 4 batch blocks of 128 rows
NJ = N // 128             # 4 j-chunks
LO, HI = 32, 16           # class split: c = 32*hi + lo

F32 = mybir.dt.float32
BF16 = mybir.dt.bfloat16
I16 = mybir.dt.int16
I32 = mybir.dt.int32
AF = mybir.ActivationFunctionType
ALU = mybir.AluOpType

STAGE = int(os.environ.get("K_STAGE", "99"))  # debug: stop after stage N


def build_program():
    nc = bacc.Bacc("TRN2", target_bir_lowering=False, debug=False,
                   num_devices=NCORES)

    x32 = nc.dram_tensor("x", [BS, N], I32, kind="ExternalInput")
    wphi = nc.dram_tensor("wphi", [C, PHI], F32, kind="ExternalInput")
    bphi = nc.dram_tensor("bphi", [PHI, 1], F32, kind="ExternalInput")
    w1 = nc.dram_tensor("w1", [PHI, H1], F32, kind="ExternalInput")
    b1 = nc.dram_tensor("b1", [PHI, H1 // PHI], F32, kind="ExternalInput")
    w2 = nc.dram_tensor("w2", [H1, H2], F32, kind="ExternalInput")
    b2 = nc.dram_tensor("b2", [PHI, H2 // PHI], F32, kind="ExternalInput")
    w3 = nc.dram_tensor("w3", [PHI, H2 // PHI], F32, kind="ExternalInput")
    b3 = nc.dram_tensor("b3", [1, 1], F32, kind="ExternalInput")
    out = nc.dram_tensor("out", [1, BS], F32, kind="ExternalOutput")

    with tile.TileContext(nc) as tc:
        with ExitStack() as ctx:
            _emit(ctx, tc, nc, x32, wphi, bphi, w1, b1, w2, b2, w3, b3, out)
    nc.compile()
    return nc


def _emit(ctx, tc, nc, x32, wphi, bphi, w1, b1, w2, b2, w3, b3, out):
    consts = ctx.enter_context(tc.tile_pool(name="consts", bufs=1))
    xin = ctx.enter_context(tc.tile_pool(name="xin", bufs=2))
    xtp = ctx.enter_context(tc.tile_pool(name="xtp", bufs=1))
    eqp = ctx.enter_context(tc.tile_pool(name="eqp", bufs=2))
    fp = ctx.enter_context(tc.tile_pool(name="fp", bufs=1))
    mlp = ctx.enter_context(tc.tile_pool(name="mlp", bufs=1))
    ps_cnt = ctx.enter_context(tc.tile_pool(name="ps_cnt", bufs=2, space="PSUM"))
    ps_u = ctx.enter_context(tc.tile_pool(name="ps_u", bufs=1, space="PSUM"))
    ps_mlp = ctx.enter_context(tc.tile_pool(name="ps_mlp", bufs=2, space="PSUM"))
    ps_y = ctx.enter_context(tc.tile_pool(name="ps_y", bufs=1, space="PSUM"))

    ident = consts.tile([128, 128], F32)
    masks.make_identity(nc, ident[:])

    # ---- weights / biases to SBUF ----
    # wphiP: W_phi replicated 4x along partitions: partition (i*32+r) holds
    # row W_phi[h*32+r] at free slot h (h=hi class). The projection matmul
    # contracts all 128 partitions at once, summing the 4 j-chunk partials.
    wphiP = consts.tile([128, HI * PHI], F32)
    wsrc = wphi.ap().rearrange("(h r) d -> r h d", r=32)
    for i in range(4):
        nc.sync.dma_start(wphiP[32 * i:32 * (i + 1), :], wsrc)
    # bf16x2 decomposition of the table for exact-ish bf16 matmuls
    wphiH = consts.tile([128, HI * PHI], BF16)
    wphiL = consts.tile([128, HI * PHI], BF16)
    wres = consts.tile([128, HI * PHI], F32)
    nc.vector.tensor_copy(wphiH[:], wphiP[:])
    nc.vector.tensor_tensor(out=wres[:], in0=wphiP[:], in1=wphiH[:],
                            op=ALU.subtract)
    nc.vector.tensor_copy(wphiL[:], wres[:])

    bphi_sb = consts.tile([128, 1], F32)
    nc.sync.dma_start(bphi_sb[:], bphi.ap())
    bphiN = consts.tile([128, 1], F32)
    nc.vector.tensor_scalar(out=bphiN[:], in0=bphi_sb[:], scalar1=float(N),
                            scalar2=None, op0=ALU.mult)

    w1sb = consts.tile([128, H1], F32)
    nc.sync.dma_start(w1sb[:], w1.ap())
    b1sb = consts.tile([128, 4], F32)
    nc.sync.dma_start(b1sb[:], b1.ap())
    w2sb = consts.tile([128, 4 * H2], F32)
    nc.sync.dma_start(w2sb[:], w2.ap().rearrange("(c p) h -> p c h", p=128))
    b2sb = consts.tile([128, 2], F32)
    nc.sync.dma_start(b2sb[:], b2.ap())
    w3sb = consts.tile([128, 2], F32)
    nc.sync.dma_start(w3sb[:], w3.ap())
    b3sb = consts.tile([1, 1], F32)
    nc.sync.dma_start(b3sb[:], b3.ap())

    # ---- index staging ----
    # xiT/xhiT/xloT: [j, (bb, jc, b)] so each block's slice is contiguous
    xiT = xtp.tile([128, NB * NJ * 128], I16)
    xhiT = xtp.tile([128, NB * NJ * 128], I16)
    xloT = xtp.tile([128, NB * NJ * 128], I16)
    # F: per-row joint counts, partition (i*32+lo), free (hi, b)
    fcnt = fp.tile([128, HI * BS], BF16)

    usb = mlp.tile([128, BS], F32)
    h1sb = [mlp.tile([128, BS], F32, tag=f"h1_{k}", name=f"h1sb{k}")
            for k in range(4)]
    h2sb = [mlp.tile([128, BS], F32, tag=f"h2_{k}", name=f"h2sb{k}")
            for k in range(2)]
    ysb = mlp.tile([1, BS], F32)

    def dbg_out(src_f32_row):
        nc.vector.tensor_copy(ysb[:], src_f32_row)
        nc.sync.dma_start(out.ap(), ysb[:])

    if STAGE == 0:
        t0 = mlp.tile([1, BS], F32, name="dbg0")
        nc.vector.tensor_copy(t0[:], wphiP[0:1, 0:BS])
        dbg_out(t0[:])
        return

    for bb in range(NB):
        # --- stage A: load 128 rows, cast, transpose, split hi/lo ---
        xrows = xin.tile([128, N], I32, tag="xrows")
        nc.sync.dma_start(xrows[:], x32.ap()[bb * 128:(bb + 1) * 128, :])
        xf = xin.tile([128, N], F32, tag="xf")
        nc.vector.tensor_copy(xf[:], xrows[:])
        for jc in range(NJ):
            pst = ps_mlp.tile([128, BS], F32, tag="ph", name="pst")
            nc.tensor.transpose(pst[:, 0:128], xf[:, jc * 128:(jc + 1) * 128],
                                ident[:])
            col = (bb * NJ + jc) * 128
            nc.vector.tensor_copy(xiT[:, col:col + 128], pst[:, 0:128])
        blk = slice(bb * 512, (bb + 1) * 512)
        nc.vector.tensor_scalar(out=xhiT[:, blk], in0=xiT[:, blk], scalar1=5,
                                scalar2=None, op0=ALU.logical_shift_right)
        nc.vector.tensor_scalar(out=xloT[:, blk], in0=xiT[:, blk], scalar1=31,
                                scalar2=None, op0=ALU.bitwise_and)
        if STAGE == 1:
            t1 = mlp.tile([1, BS], F32, name="dbg1")
            nc.vector.tensor_copy(t1[:], xloT[0:1, :BS])
            dbg_out(t1[:])
            return

        # --- stage B: one-hots via is_equal ---
        # H2 [j, (jc, lo, b)]  G2 [j, (jc, hi, b)] — per-row matmul operand
        # slices are then single stride-128 runs (walrus requires matmul
        # APs with one free dim), eq writes keep a packed innermost dim.
        h2t = eqp.tile([128, LO * NJ * 128], BF16, tag="h2t")
        g2t = eqp.tile([128, HI * NJ * 128], BF16, tag="g2t")
        h2v = h2t[:].rearrange("p (jc l b) -> p jc l b", jc=NJ, l=LO)
        g2v = g2t[:].rearrange("p (jc h b) -> p jc h b", jc=NJ, h=HI)
        for lo in range(LO):
            nc.vector.tensor_scalar(out=h2v[:, :, lo:lo + 1, :],
                                    in0=xloT[:, blk], scalar1=lo,
                                    scalar2=None, op0=ALU.is_equal)
        for hi in range(HI):
            nc.vector.tensor_scalar(out=g2v[:, :, hi:hi + 1, :],
                                    in0=xhiT[:, blk], scalar1=hi,
                                    scalar2=None, op0=ALU.is_equal)
        if STAGE == 2:
            t2 = mlp.tile([1, BS], F32, name="dbg2")
            nc.vector.tensor_copy(t2[:], h2t[0:1, :BS])
            dbg_out(t2[:])
            return

        # --- stage C: per-row count matmuls ---
        # one matmul per row: lhsT [j, (jc, lo)] (m=128), rhs [j, (jc', hi)]
        # (n=64); diagonal jc==jc' blocks of out hold the counts.
        h2m = h2t[:].rearrange("p (m b) -> p m b", b=128)
        g2m = g2t[:].rearrange("p (m b) -> p m b", b=128)
        fv = fcnt[:].rearrange("p (h b) -> p h b", h=HI)
        for k16 in range(8):            # 8 psum tiles of 16 rows each
            pc = ps_cnt.tile([128, 1024], F32)
            for s in range(16):
                b_l = k16 * 16 + s
                nc.tensor.matmul(
                    pc[:, s * 64:(s + 1) * 64],
                    h2m[:, :, b_l:b_l + 1],
                    g2m[:, :, b_l:b_l + 1],
                    start=True, stop=True)
            # evacuate diagonal blocks to F (ACT, Copy only -> no table swaps)
            pcv = pc[:].rearrange("p (s i h) -> p s i h", s=16, i=NJ)
            b0 = bb * 128 + k16 * 16
            for i in range(NJ):
                src = pcv[32 * i:32 * (i + 1), :, i:i + 1, :]
                dst = fv[32 * i:32 * (i + 1), :, b0:b0 + 16]
                nc.scalar.copy(dst.transpose([0, 2, 1]), src)

    if STAGE == 3:
        t3 = mlp.tile([1, BS], F32, name="dbg3")
        nc.vector.tensor_copy(t3[:], fcnt[0:1, :BS])
        dbg_out(t3[:])
        return

    # ---- projection u_T[d, b] = sum_c counts_T[c, b] * W_phi[c, d] ----
    # Weights replicated across the 4 partition blocks: one k=128 matmul per
    # hi-class sums over lo-classes and the 4 j-chunk partials.
    pu = ps_u.tile([128, BS], F32)
    k = 0
    for h in range(HI):
        for w in (wphiH, wphiL):
            nc.tensor.matmul(
                pu[:], w[:, PHI * h:PHI * (h + 1)], fv[:, h, :],
                start=(k == 0), stop=(k == 2 * HI - 1))
            k += 1
    nc.vector.tensor_scalar(out=usb[:], in0=pu[:], scalar1=bphiN[:, 0:1],
                            scalar2=None, op0=ALU.add)
    if STAGE == 4:
        dbg_out(usb[0:1, :])
        return

    # ---- MLP ----
    for hc in range(4):
        ph = ps_mlp.tile([128, BS], F32, tag="ph", name="ph_a")
        nc.tensor.matmul(ph[:], w1sb[:, hc * 128:(hc + 1) * 128], usb[:],
                         start=True, stop=True)
        nc.scalar.activation(h1sb[hc][:], ph[:], AF.Relu,
                             bias=b1sb[:, hc:hc + 1], scale=1.0)
    w2v = w2sb[:].rearrange("p (c h) -> p c h", c=4)
    for mc in range(2):
        ph = ps_mlp.tile([128, BS], F32, tag="ph", name="ph_b")
        for kc in range(4):
            nc.tensor.matmul(ph[:], w2v[:, kc, mc * 128:(mc + 1) * 128],
                             h1sb[kc][:], start=(kc == 0), stop=(kc == 3))
        nc.scalar.activation(h2sb[mc][:], ph[:], AF.Relu,
                             bias=b2sb[:, mc:mc + 1], scale=1.0)
    py = ps_y.tile([1, BS], F32)
    for kc in range(2):
        nc.tensor.matmul(py[:], w3sb[:, kc:kc + 1], h2sb[kc][:],
                         start=(kc == 0), stop=(kc == 1))
    nc.vector.tensor_scalar(out=ysb[:], in0=py[:], scalar1=b3sb[0:1, 0:1],
                            scalar2=None, op0=ALU.add)
    nc.sync.dma_start(out.ap(), ysb[:])


_CACHED_NC = None


def _get_nc():
    global _CACHED_NC
    if _CACHED_NC is None:
        _CACHED_NC = build_program()
    return _CACHED_NC


def _prep_in_maps(x, W_phi, b_phi, W1, b1, W2, b2, W3, b3):
    x = np.ascontiguousarray(np.asarray(x, dtype=np.int64).astype(np.int32))
    W_phi = np.asarray(W_phi, dtype=np.float32)
    W1 = np.asarray(W1, dtype=np.float32)
    W2 = np.asarray(W2, dtype=np.float32)
    shared = {
        "wphi": W_phi,
        "bphi": np.asarray(b_phi, dtype=np.float32).reshape(PHI, 1),
        "w1": W1,
        "b1": np.ascontiguousarray(
            np.asarray(b1, np.float32).reshape(4, 128).T),
        "w2": W2,
        "b2": np.ascontiguousarray(
            np.asarray(b2, np.float32).reshape(2, 128).T),
        "w3": np.ascontiguousarray(
            np.asarray(W3, np.float32).reshape(2, 128).T),
        "b3": np.asarray(b3, np.float32).reshape(1, 1),
    }
    return [dict(shared, x=np.ascontiguousarray(x[c * BS:(c + 1) * BS]))
            for c in range(NCORES)]


def run(trace=False, tmpdir=None, **inputs):
    nc = _get_nc()
    in_maps = _prep_in_maps(**inputs)
    res = run_bass_kernel_spmd(nc, in_maps, core_ids=list(range(NCORES)),
                               trace=trace, tmpdir=tmpdir)
    y = np.concatenate([np.asarray(res.results[c]["out"]).reshape(BS)
                        for c in range(NCORES)])
    return y.reshape(B, 1).astype(np.float32), res


def kernel(**inputs):
    y, _ = run(trace=False, **inputs)
    return y



# revision 3
# speedup vs baseline: 5.3240x; 1.0041x over previous
"""Trainium2 Bass kernel v2 for DeepSet MLP (embedding-lookup-sum + MLP).

Math: u[b] = sum_j W_phi[x[b,j]] + N*b_phi; y = relu(relu(u@W1+b1)@W2+b2)@W3+b3.

Scheme (per core, 512 rows, data-parallel over 8 cores):
  - Class split c = 32*hi + lo. Host sends pre-transposed split index
    tensors xloQ/xhiQ (bf16) in quad layout [j%128, (blk4, i4, bq32, r4)].
  - One-hots in quad-interleaved layout hq[p, (i, bq, lo, r)] /
    gq[p, (i, bq, h, r)] so that (a) the per-quad matmul stationary
    slice (lo,r)=128 cols is contiguous (FWL fast weight load) and
    (b) writes are 4-elem contiguous runs (TS mode) or fully dense
    (TT broadcast mode).
  - Count matmuls: per 4-row quad, 4 j-chunk matmuls accumulate in
    PSUM: out[(lo,r), (h,r')] holds complete counts on r==r' diagonal.
  - Evacuation: whole PSUM banks (8 quads) copied to SBUF bf16 in one
    contiguous ACT copy (counts are small ints: bf16-exact).
  - Projection u = counts @ W_phi via diagonal-absorbing matmuls: for
    each (r', h) a K=128 matmul whose stationary has W_phi rows (32h+lo)
    embedded at partitions (lo, r') and zeros elsewhere.
  - MLP in bf16 with mean-centered activations: the host folds the
    large constant offsets (N*b_phi path) into per-layer f32 biases and
    subtracts per-layer means so every bf16 tensor is small.
"""

import os
import numpy as np
from contextlib import ExitStack

import concourse.bass as bass
import concourse.bacc as bacc
import concourse.tile as tile
import concourse.mybir as mybir
from concourse.bass_utils import run_bass_kernel_spmd

B, N, C, PHI = 4096, 512, 512, 128
H1, H2 = 512, 256
NCORES = 8
BS = B // NCORES          # 512 rows per core
NI = 4                    # j chunks of 128
NBLK = 4                  # row blocks (32 quads each)
BQB = 32                  # quads per block
LO, HI = 32, 16

F32 = mybir.dt.float32
BF16 = mybir.dt.bfloat16
ALU = mybir.AluOpType
AF = mybir.ActivationFunctionType

ONEHOT_MODE = os.environ.get("K_ONEHOT", "ts")  # "ts" or "tt"


def build_program():
    nc = bacc.Bacc("TRN2", target_bir_lowering=False, debug=False,
                   num_devices=NCORES)

    xlo = nc.dram_tensor("xlo", [128, NBLK * NI * BQB * 4], BF16,
                         kind="ExternalInput")
    xhi = nc.dram_tensor("xhi", [128, NBLK * NI * BQB * 4], BF16,
                         kind="ExternalInput")
    iolo = nc.dram_tensor("iolo", [128, LO], BF16, kind="ExternalInput")
    iohi = nc.dram_tensor("iohi", [128, HI], BF16, kind="ExternalInput")
    wselz = nc.dram_tensor("wselz", [128, 4 * HI * PHI], BF16,
                           kind="ExternalInput")
    w1 = nc.dram_tensor("w1", [PHI, H1], BF16, kind="ExternalInput")
    b1p = nc.dram_tensor("b1p", [128, 4], F32, kind="ExternalInput")
    c1n = nc.dram_tensor("c1n", [128, 4], F32, kind="ExternalInput")
    w2 = nc.dram_tensor("w2", [128, 4 * 2 * 128], BF16, kind="ExternalInput")
    b2p = nc.dram_tensor("b2p", [128, 2], F32, kind="ExternalInput")
    c2n = nc.dram_tensor("c2n", [128, 2], F32, kind="ExternalInput")
    w3 = nc.dram_tensor("w3", [128, 2], BF16, kind="ExternalInput")
    off0 = nc.dram_tensor("off0", [128, 1], F32, kind="ExternalInput")
    b3p = nc.dram_tensor("b3p", [1, 1], F32, kind="ExternalInput")
    out = nc.dram_tensor("out", [1, BS], F32, kind="ExternalOutput")

    with tile.TileContext(nc) as tc:
        with ExitStack() as ctx:
            _emit(ctx, tc, nc, xlo, xhi, iolo, iohi, wselz, w1, b1p, c1n,
                  w2, b2p, c2n, w3, off0, b3p, out)
    nc.compile()
    return nc


def _emit(ctx, tc, nc, xlo, xhi, iolo, iohi, wselz, w1, b1p, c1n, w2, b2p,
          c2n, w3, off0, b3p, out):
    consts = ctx.enter_context(tc.tile_pool(name="consts", bufs=1))
    ohp = ctx.enter_context(tc.tile_pool(name="ohp", bufs=2))
    mlp = ctx.enter_context(tc.tile_pool(name="mlp", bufs=1))
    xin = ctx.enter_context(tc.tile_pool(name="xin", bufs=4))
    ps_cnt = ctx.enter_context(tc.tile_pool(name="ps_cnt", bufs=2,
                                            space="PSUM"))
    ps_u = ctx.enter_context(tc.tile_pool(name="ps_u", bufs=1, space="PSUM"))
    ps_mlp = ctx.enter_context(tc.tile_pool(name="ps_mlp", bufs=2,
                                            space="PSUM"))
    ps_y = ctx.enter_context(tc.tile_pool(name="ps_y", bufs=1, space="PSUM"))

    # Index tensors first: the one-hot pipeline depends only on these, so
    # they must not queue behind the big constant DMAs.
    BLKW0 = NI * BQB * 4
    xls = [xin.tile([128, BLKW0], BF16, tag="xl", name=f"xl{b}")
           for b in range(NBLK)]
    xhs = [xin.tile([128, BLKW0], BF16, tag="xh", name=f"xh{b}")
           for b in range(NBLK)]
    for b in range(NBLK):
        nc.sync.dma_start(xls[b][:], xlo.ap()[:, b * BLKW0:(b + 1) * BLKW0])
        nc.sync.dma_start(xhs[b][:], xhi.ap()[:, b * BLKW0:(b + 1) * BLKW0])
    iolo_sb = consts.tile([128, LO], BF16)
    nc.sync.dma_start(iolo_sb[:], iolo.ap())
    iohi_sb = consts.tile([128, HI], BF16)
    nc.sync.dma_start(iohi_sb[:], iohi.ap())
    wselz_sb = consts.tile([128, 4 * HI * PHI], BF16)
    nc.sync.dma_start(wselz_sb[:], wselz.ap())
    w1sb = consts.tile([128, H1], BF16)
    nc.sync.dma_start(w1sb[:], w1.ap())
    b1psb = consts.tile([128, 4], F32)
    nc.sync.dma_start(b1psb[:], b1p.ap())
    c1nsb = consts.tile([128, 4], F32)
    nc.sync.dma_start(c1nsb[:], c1n.ap())
    w2sb = consts.tile([128, 4 * 2 * 128], BF16)
    nc.sync.dma_start(w2sb[:], w2.ap())
    b2psb = consts.tile([128, 2], F32)
    nc.sync.dma_start(b2psb[:], b2p.ap())
    c2nsb = consts.tile([128, 2], F32)
    nc.sync.dma_start(c2nsb[:], c2n.ap())
    w3sb = consts.tile([128, 2], BF16)
    nc.sync.dma_start(w3sb[:], w3.ap())
    off0sb = consts.tile([128, 1], F32)
    nc.sync.dma_start(off0sb[:], off0.ap())
    b3psb = consts.tile([1, 1], F32)
    nc.sync.dma_start(b3psb[:], b3p.ap())

    # counts: [128=(lo32,r4), (h16, r'4, bq128)] bf16 — h/r'-major so the
    # projection's moving-operand columns are contiguous
    fvq = mlp.tile([128, 128 * HI * 4], BF16)
    fvh = fvq[:].rearrange("p (h r q) -> p h r q", h=HI, r=4)

    for blk in range(NBLK):
        xl = xls[blk][:]
        xh = xhs[blk][:]

        hq = ohp.tile([128, NI * BQB * LO * 4], BF16, tag="hq")
        gq = ohp.tile([128, NI * BQB * HI * 4], BF16, tag="gq")
        hqv = hq[:].rearrange("p (i q l r) -> p i q l r", i=NI, q=BQB, l=LO)
        gqv = gq[:].rearrange("p (i q l r) -> p i q l r", i=NI, q=BQB, l=HI)
        xlv = xl.rearrange("p (i q r) -> p i q r", i=NI, q=BQB)
        xhv = xh.rearrange("p (i q r) -> p i q r", i=NI, q=BQB)
        del xl, xh

        if ONEHOT_MODE == "ts":
            for lo in range(LO):
                nc.vector.tensor_scalar(out=hqv[:, :, :, lo, :], in0=xlv,
                                        scalar1=lo, scalar2=None,
                                        op0=ALU.is_equal)
            for h in range(HI):
                nc.vector.tensor_scalar(out=gqv[:, :, :, h, :], in0=xhv,
                                        scalar1=h, scalar2=None,
                                        op0=ALU.is_equal)
        else:
            in0l = xlv.unsqueeze(3).broadcast_to([128, NI, BQB, LO, 4])
            in1l = iolo_sb[:].unsqueeze(1).unsqueeze(1).unsqueeze(4) \
                .broadcast_to([128, NI, BQB, LO, 4])
            nc.vector.tensor_tensor(out=hqv, in0=in0l, in1=in1l,
                                    op=ALU.is_equal)
            in0h = xhv.unsqueeze(3).broadcast_to([128, NI, BQB, HI, 4])
            in1h = iohi_sb[:, :HI].unsqueeze(1).unsqueeze(1).unsqueeze(4) \
                .broadcast_to([128, NI, BQB, HI, 4])
            nc.vector.tensor_tensor(out=gqv, in0=in0h, in1=in1h,
                                    op=ALU.is_equal)

        hqm = hq[:].rearrange("p (i q c) -> p i q c", i=NI, q=BQB)
        gqm = gq[:].rearrange("p (i q c) -> p i q c", i=NI, q=BQB)

        for t in range(4):
            pt = ps_cnt.tile([128, 512], F32, tag="cnt")
            for q8 in range(8):
                qb = t * 8 + q8
                for i in range(NI):
                    nc.tensor.matmul(
                        pt[:, q8 * 64:(q8 + 1) * 64],
                        hqm[:, i, qb, :],
                        gqm[:, i, qb, :],
                        start=(i == 0), stop=(i == NI - 1))
            # transposing evacuation: psum (q8, h, r') -> fvq (h, r', bq);
            # dst writes are 8-elem (16B) contiguous runs
            bq0 = (blk * 4 + t) * 8
            dst = fvh[:, :, :, bq0:bq0 + 8]
            src = pt[:].rearrange("p (q h r) -> p h r q", q=8, h=HI)
            nc.scalar.copy(dst, src)

    # ---- projection with diagonal-absorbing zero-padded stationaries ----
    # u columns in (r, k) order: each r-chain writes a contiguous 128-col
    # PSUM region (stride-4 PSUM writes quadruple the matmul drain time).
    # The b-order is restored by the output DMA's read pattern.
    pu = ps_u.tile([128, BS], F32)
    for r in range(4):
        for h in range(HI):
            nc.tensor.matmul(
                pu[:, r * 128:(r + 1) * 128],
                wselz_sb[:, (r * HI + h) * PHI:(r * HI + h + 1) * PHI],
                fvh[:, h, r, :],
                start=(h == 0), stop=(h == HI - 1))
    usb = mlp.tile([128, BS], BF16)
    nc.vector.tensor_scalar(out=usb[:], in0=pu[:], scalar1=off0sb[:, 0:1],
                            scalar2=None, op0=ALU.add)

    # ---- MLP (bf16, centered) ----
    h1sb = [mlp.tile([128, BS], BF16, tag=f"h1_{k}", name=f"h1sb{k}")
            for k in range(4)]
    h2sb = [mlp.tile([128, BS], BF16, tag=f"h2_{k}", name=f"h2sb{k}")
            for k in range(2)]
    h1t = [mlp.tile([128, BS], F32, tag=f"h1t_{k}", name=f"h1t{k}")
           for k in range(4)]
    h2t = [mlp.tile([128, BS], F32, tag=f"h2t_{k}", name=f"h2t{k}")
           for k in range(2)]

    for hc in range(4):
        ph = ps_mlp.tile([128, BS], F32, tag="ph", name="ph_a")
        nc.tensor.matmul(ph[:], w1sb[:, hc * 128:(hc + 1) * 128], usb[:],
                         start=True, stop=True)
        nc.scalar.activation(h1t[hc][:], ph[:], AF.Relu,
                             bias=b1psb[:, hc:hc + 1], scale=1.0)
        nc.vector.tensor_scalar(out=h1sb[hc][:], in0=h1t[hc][:],
                                scalar1=c1nsb[:, hc:hc + 1], scalar2=None,
                                op0=ALU.add)
    w2v = w2sb[:].rearrange("p (kc m) -> p kc m", kc=4)
    for mc in range(2):
        ph = ps_mlp.tile([128, BS], F32, tag="ph", name="ph_b")
        for kc in range(4):
            nc.tensor.matmul(ph[:], w2v[:, kc, mc * 128:(mc + 1) * 128],
                             h1sb[kc][:], start=(kc == 0), stop=(kc == 3))
        nc.scalar.activation(h2t[mc][:], ph[:], AF.Relu,
                             bias=b2psb[:, mc:mc + 1], scale=1.0)
        nc.vector.tensor_scalar(out=h2sb[mc][:], in0=h2t[mc][:],
                                scalar1=c2nsb[:, mc:mc + 1], scalar2=None,
                                op0=ALU.add)
    py = ps_y.tile([1, BS], F32)
    for kc in range(2):
        nc.tensor.matmul(py[:], w3sb[:, kc:kc + 1], h2sb[kc][:],
                         start=(kc == 0), stop=(kc == 1))
    ysb = mlp.tile([1, BS], F32)
    nc.vector.tensor_scalar(out=ysb[:], in0=py[:], scalar1=b3psb[0:1, 0:1],
                            scalar2=None, op0=ALU.add)
    # ysb columns are in (r, k) order; host un-permutes (b = k*4+r)
    nc.sync.dma_start(out.ap(), ysb[:])


_CACHED_NC = None


def _get_nc():
    global _CACHED_NC
    if _CACHED_NC is None:
        _CACHED_NC = build_program()
    return _CACHED_NC


def _prep_in_maps(x, W_phi, b_phi, W1, b1, W2, b2, W3, b3):
    import ml_dtypes
    bf = ml_dtypes.bfloat16

    x = np.asarray(x, dtype=np.int64)
    Wd = np.asarray(W_phi, np.float64)
    bphid = np.asarray(b_phi, np.float64)
    W1d = np.asarray(W1, np.float64)
    W2d = np.asarray(W2, np.float64)
    W3d = np.asarray(W3, np.float64)
    b1d, b2d, b3d = (np.asarray(v, np.float64) for v in (b1, b2, b3))

    Wb = Wd.astype(np.float32).astype(bf).astype(np.float64)
    ubar = (N / C) * Wb.sum(0) + N * bphid
    off0v = (N * bphid - ubar).astype(np.float32)
    b1pv = (b1d + ubar @ W1d).astype(np.float32)
    c1v = np.maximum(b1pv, 0.0)
    b2pv = (b2d + c1v.astype(np.float64) @ W2d).astype(np.float32)
    c2v = np.maximum(b2pv, 0.0)
    b3pv = (b3d + c2v.astype(np.float64) @ W3d).astype(np.float32)

    # wselz[(lo,r) partition, (rv, h, d)] = Wb[32h+lo, d] iff r == rv
    wsel = Wb.astype(np.float32).reshape(HI, LO, PHI)     # [h, lo, d]
    wz = np.zeros((LO, 4, 4, HI, PHI), np.float32)        # [lo, r, rv, h, d]
    for r in range(4):
        wz[:, r, r, :, :] = wsel.transpose(1, 0, 2)
    wselzv = np.ascontiguousarray(
        wz.reshape(128, 4 * HI * PHI).astype(bf))

    w1v = np.ascontiguousarray(W1d.astype(np.float32).astype(bf))
    w2vv = np.ascontiguousarray(
        W2d.astype(np.float32).reshape(4, 128, 2, 128).transpose(1, 0, 2, 3)
        .reshape(128, 1024).astype(bf))
    w3v = np.ascontiguousarray(
        W3d.astype(np.float32).reshape(2, 128).T.astype(bf))

    shared = {
        "iolo": np.ascontiguousarray(np.broadcast_to(
            np.arange(LO, dtype=np.float32), (128, LO)).astype(bf)),
        "iohi": np.ascontiguousarray(np.broadcast_to(
            np.arange(HI, dtype=np.float32), (128, HI)).astype(bf)),
        "wselz": wselzv,
        "w1": w1v,
        "b1p": np.ascontiguousarray(b1pv.reshape(4, 128).T),
        "c1n": np.ascontiguousarray((-c1v).reshape(4, 128).T),
        "w2": w2vv,
        "b2p": np.ascontiguousarray(b2pv.reshape(2, 128).T),
        "c2n": np.ascontiguousarray((-c2v).reshape(2, 128).T),
        "w3": w3v,
        "off0": np.ascontiguousarray(off0v.reshape(128, 1)),
        "b3p": np.ascontiguousarray(b3pv.reshape(1, 1)),
    }

    lo_all = (x & 31).astype(np.float32)
    hi_all = (x >> 5).astype(np.float32)

    def quadpack(v):  # [512b, 512j] -> [128p, (blk, i, bq, r)]
        a = v.T.reshape(NI, 128, NBLK, BQB, 4)       # [i, p, blk, bq, r]
        a = a.transpose(1, 2, 0, 3, 4)               # [p, blk, i, bq, r]
        return np.ascontiguousarray(a.reshape(128, -1).astype(bf))

    maps = []
    for c in range(NCORES):
        sl = slice(c * BS, (c + 1) * BS)
        m = dict(shared)
        m["xlo"] = quadpack(lo_all[sl])
        m["xhi"] = quadpack(hi_all[sl])
        maps.append(m)
    return maps


def run(trace=False, tmpdir=None, **inputs):
    nc = _get_nc()
    in_maps = _prep_in_maps(**inputs)
    res = run_bass_kernel_spmd(nc, in_maps, core_ids=list(range(NCORES)),
                               trace=trace, tmpdir=tmpdir)
    # device emits y in (r, k) order per core; un-permute to b = k*4 + r
    y = np.concatenate([
        np.asarray(res.results[c]["out"]).reshape(4, BS // 4).T.reshape(BS)
        for c in range(NCORES)])
    return y.reshape(B, 1).astype(np.float32), res


def kernel(**inputs):
    y, _ = run(trace=False, **inputs)
    return y
